# revision 24
# baseline (speedup 1.0000x reference)
"""Local (sliding-window) self-attention Bass kernel for 8 TRN2 NeuronCores.

Problem: B=4, T=4096, C=512, H=8 heads, head_dim=64, window=15.
Sharding: 8 cores = batch(4) x seq-halves(2). Each core processes 2048 query
tokens of one batch element; its x chunk carries a 7-token halo on each side
(zero-padded at sequence edges, matching the reference's jnp.pad semantics),
padded to 2080 rows for 128/32 alignment.

v2 dataflow (no DMA transposes; scores computed k-major so alpha never needs
transposing; per-128-token-block batching of ACT/DVE work):
  x chunk --mask*cast--> x_tok bf16 --PE transpose--> xT (feature-major)
  qT/kT = W-stationary matmuls + bias (feature-major)
  v_tok = xT-stationary matmuls + bias, token-major, packed [64 v | 1 ones]
          per head (the ones column makes AV emit the softmax denominator)
  per 128-query block:
    scoresT [k,q] = kT-slice.T @ qT-slice per head (A:128k + B:16k chunks)
    one Exp over the whole block's scores -> alphaT bf16
    one DVE band-mask multiply
    AV token-major: av[q, 65] per head = alphaT.T @ v_aug  (den in col 64)
    reciprocal + one DVE normalize (rden broadcast via 0-stride AP) -> bf16
    4 PE transposes -> attnT feature-major -> one ACT copy -> aT
    proj: aT-stationary matmuls + bias + mask -> DMA out (token-major)
"""

import math
import os
from contextlib import ExitStack

import ml_dtypes
import numpy as np

import concourse.bacc as bacc
import concourse.bass as bass
import concourse.mybir as mybir
import concourse.tile as tile
from concourse import bass_utils

B, T, C, H, WIN = 4, 4096, 512, 8, 15
D = C // H            # 64
PAD = WIN // 2        # 7
NTOK = T // 2         # 2048 query tokens per core
NKV = 2112            # kv rows per core: 7 + 2048 + 7 = 2062, padded to 2112
                      # (the per-block k-tail chunk is 64 wide: 16*128+64)
NB = NTOK // 128      # 16 query blocks
SCALE = math.log(WIN) / D
F32 = mybir.dt.float32
BF16 = mybir.dt.bfloat16
FP8 = mybir.dt.float8e4
WSC = 16.0           # fp8 weight pre-scale; 1/WSC folded into psum->SBUF copies


M0 = 2000.0   # additive score mask; SCALE*M0 ~ 85 so exp underflows to 0


def _mask_consts() -> dict:
    """Additive band masks (transposed, as matmul lhsT) and a replicated
    identity used to broadcast them across the 4 head-columns of a bank.

    A-chunk in-band: k-14 <= q <= k.  B-chunk (k=128+r): q >= 114+r.
    """
    k = np.arange(128)[:, None]
    q = np.arange(128)[None, :]
    a = np.where((q >= k - 14) & (q <= k), 0.0, -M0).astype(np.float32)
    # B tail: k' = 0..63 (k = qb+128+k'), q64 = 0..63 (q = qb+64+q64):
    # in-band iff q >= 114 + k'  <=>  q64 >= 50 + k'
    kp = np.arange(64)[:, None]
    q64 = np.arange(64)[None, :]
    b = np.where(q64 >= 50 + kp, 0.0, -M0).astype(np.float32)
    identj = np.zeros((128, 512), np.float32)
    for j in range(4):
        identj[:, j * 128:(j + 1) * 128] = np.eye(128)
    # K=128 operands (zero-padded rows 64..127) so the B-mask matmul shares
    # the full-array row config of the A-mask (base-mix rule)
    identj32 = np.zeros((128, 256), np.float32)
    for j in range(4):
        identj32[0:64, j * 64:(j + 1) * 64] = np.eye(64)
    return {
        "bandat": np.ascontiguousarray(a.T.astype(ml_dtypes.bfloat16)),
        "bandbt": np.ascontiguousarray(
            np.vstack([b.T, np.zeros((64, 64), np.float32)])
              .astype(ml_dtypes.bfloat16)),
        "identj": np.ascontiguousarray(identj.astype(ml_dtypes.bfloat16)),
        "identj32": np.ascontiguousarray(identj32.astype(ml_dtypes.bfloat16)),
    }


def _identity() -> np.ndarray:
    return np.eye(128, dtype=ml_dtypes.bfloat16)


def build_program() -> bacc.Bacc:
    nc = bacc.Bacc("TRN2", target_bir_lowering=False, debug=False,
                   enable_asserts=False, num_devices=8)

    xd = nc.dram_tensor("x", [NKV, C], F32, kind="ExternalInput").ap()
    maskd = nc.dram_tensor("mask", [NKV], F32, kind="ExternalInput").ap()
    wqd = nc.dram_tensor("wq", [C, C], F32, kind="ExternalInput").ap()
    bqd = nc.dram_tensor("bq", [C], F32, kind="ExternalInput").ap()
    wkvd = nc.dram_tensor("wkv", [C, 2 * C], F32, kind="ExternalInput").ap()
    bkvd = nc.dram_tensor("bkv", [2 * C], F32, kind="ExternalInput").ap()
    wpd = nc.dram_tensor("wproj", [C, C], F32, kind="ExternalInput").ap()
    bpd = nc.dram_tensor("bproj", [C], F32, kind="ExternalInput").ap()
    bandatd = nc.dram_tensor("bandat", [128, 128], BF16, kind="ExternalInput").ap()
    bandbtd = nc.dram_tensor("bandbt", [128, 64], BF16, kind="ExternalInput").ap()
    identjd = nc.dram_tensor("identj", [128, 512], BF16, kind="ExternalInput").ap()
    identj32d = nc.dram_tensor("identj32", [128, 256], BF16, kind="ExternalInput").ap()
    identd = nc.dram_tensor("ident", [128, 128], BF16, kind="ExternalInput").ap()
    outd = nc.dram_tensor("out", [NTOK, C], F32, kind="ExternalOutput").ap()

    with tile.TileContext(nc) as tc, ExitStack() as ctx:
        sb = ctx.enter_context(tc.tile_pool(name="sb", bufs=1))
        sb_ld = ctx.enter_context(tc.tile_pool(name="sb_ld", bufs=3))
        sb_a = ctx.enter_context(tc.tile_pool(name="sb_a", bufs=3))
        sb_o = ctx.enter_context(tc.tile_pool(name="sb_o", bufs=3))
        pp_sc = ctx.enter_context(tc.tile_pool(name="pp_sc", bufs=1, space="PSUM"))
        pp_tr = ctx.enter_context(tc.tile_pool(name="pp_tr", bufs=2, space="PSUM"))
        pp_av = ctx.enter_context(tc.tile_pool(name="pp_av", bufs=1, space="PSUM"))

        # ---- persistent SBUF tensors ----
        xT = sb.tile([128, 4 * NKV], BF16, tag="xT")     # col ci*NKV + t
        qT = sb.tile([128, 4 * NTOK], BF16, tag="qT")    # col co*NTOK + t
        kT = sb.tile([128, 4 * NKV], BF16, tag="kT")     # col co*NKV + t
        aT = sb.tile([128, 4 * NTOK], BF16, tag="aT")    # col ct*NTOK + q
        v_tok = [sb.tile([128, 520], BF16, tag=f"vtok{i}", name=f"vtok{i}")
                 for i in range(17)]                     # col h*65: [64 v | 1]
        bandat = sb.tile([128, 128], BF16, tag="bandat")
        bandbt = sb.tile([128, 64], BF16, tag="bandbt")
        identj = sb.tile([128, 512], BF16, tag="identj")
        identj32 = sb.tile([128, 256], BF16, tag="identj32")
        ident = sb.tile([128, 128], BF16, tag="ident")
        mqr = sb.tile([1, NTOK], BF16, tag="mqr")     # mask row (proj bias fold)
        bpr = sb.tile([1, C], BF16, tag="bpr")        # bproj row
        wq = [sb.tile([128, C], BF16, tag=f"wq{i}", name=f"wq{i}") for i in range(4)]
        wk = [sb.tile([128, C], BF16, tag=f"wk{i}", name=f"wk{i}") for i in range(4)]
        wv = [sb.tile([128, C], BF16, tag=f"wv{i}", name=f"wv{i}") for i in range(4)]
        wp = [sb.tile([128, C], BF16, tag=f"wp{i}", name=f"wp{i}") for i in range(4)]
        bq_t = sb.tile([128, 4], F32, tag="bq")       # per-partition q bias
        bk_t = sb.tile([128, 4], F32, tag="bk")       # per-partition k bias
        bvB = sb.tile([128, C], F32, tag="bvB")       # v bias bcast over partitions
        mq = sb.tile([128, NB], F32, tag="mq")        # query-token mask, per block

        # ---- constants / weights in ----
        nc.sync.dma_start(bandat[:], bandatd)
        nc.sync.dma_start(bandbt[:], bandbtd)
        nc.sync.dma_start(identj[:], identjd)
        nc.sync.dma_start(identj32[:], identj32d)
        nc.sync.dma_start(ident[:], identd)
        mqf = sb_ld.tile([1, NTOK], F32, tag="mqf")
        nc.sync.dma_start(mqf[:], maskd[PAD:PAD + NTOK][None, :])
        nc.vector.tensor_copy(mqr[:], mqf[:])
        bprf = sb_ld.tile([1, C], F32, tag="bprf")
        nc.sync.dma_start(bprf[:], bpd[None, :])
        nc.vector.tensor_copy(bpr[:], bprf[:])
        nc.sync.dma_start(bq_t[:], bqd.rearrange("(a b) -> b a", b=128))
        nc.sync.dma_start(bk_t[:], bkvd[0:C].rearrange("(a b) -> b a", b=128))
        nc.sync.dma_start(bvB[:], bkvd[C:2 * C][None, :].broadcast_to((128, C)))
        nc.sync.dma_start(mq[:], maskd[PAD:PAD + NTOK].rearrange("(a b) -> b a", b=128))
        for ci in range(4):
            wqf = sb_ld.tile([128, C], F32, tag="wld")
            nc.sync.dma_start(wqf[:], wqd[ci * 128:(ci + 1) * 128, :])
            nc.vector.tensor_copy(wq[ci][:], wqf[:])
            wkf = sb_ld.tile([128, 2 * C], F32, tag="wld2")
            nc.sync.dma_start(wkf[:], wkvd[ci * 128:(ci + 1) * 128, :])
            nc.vector.tensor_copy(wk[ci][:], wkf[:, 0:C])
            nc.vector.tensor_copy(wv[ci][:], wkf[:, C:2 * C])
            wpf = sb_ld.tile([128, C], F32, tag="wld")
            nc.sync.dma_start(wpf[:], wpd[ci * 128:(ci + 1) * 128, :])
            nc.vector.tensor_copy(wp[ci][:], wpf[:])

        # ---- x in: mask*cast, then PE-transpose to feature-major ----
        for t in range(17):
            r0, r1 = t * 128, min((t + 1) * 128, NKV)
            rows = r1 - r0
            xf = sb_ld.tile([128, C], F32, tag="xf")
            nc.sync.dma_start(xf[:rows, :], xd[r0:r1, :])
            mrow = sb_ld.tile([128, 1], F32, tag="mrow")
            nc.sync.dma_start(mrow[:rows, :], maskd[r0:r1][:, None])
            xb = sb_ld.tile([128, C], BF16, tag="xb")
            nc.vector.tensor_scalar_mul(xb[:rows, :], xf[:rows, :], mrow[:rows, :])
            xtr = pp_tr.tile([128, 512], BF16, tag="tr", name="xtr")
            for ci in range(4):
                nc.tensor.transpose(
                    xtr[:, ci * 128:ci * 128 + rows],
                    xb[:rows, ci * 128:(ci + 1) * 128],
                    ident[:rows, :rows])
            nc.scalar.activation(
                xT.rearrange("p (a c) -> p a c", a=4)[:, :, r0:r1],
                xtr.rearrange("p (a c) -> p a c", a=4)[:, :, 0:rows],
                mybir.ActivationFunctionType.Copy)

        # Alternate projection-phase PSUM tiles between the two big pools so
        # consecutive chunks double-buffer (each pool alone has bufs=1).
        pcnt = [0]

        def proj_ps():
            pool = pp_sc if pcnt[0] % 2 == 0 else pp_av
            pcnt[0] += 1
            shape = [128, 2048] if pool is pp_sc else [128, 1024]
            return pool.tile(shape, F32, tag="sc" if pool is pp_sc else "av",
                             name=f"pps{pcnt[0]}")

        # ---- qT (feature-major): W stationary, xT moving ----
        _P1 = 4 if int(os.environ.get("KPH", "5")) >= 1 else 0
        for co in range(_P1):
            for ch in range(4):
                t0 = ch * 512
                ps = proj_ps()
                for ci in range(4):
                    nc.tensor.matmul(
                        ps[:, 0:512], wq[ci][:, co * 128:(co + 1) * 128],
                        xT[:, ci * NKV + PAD + t0:ci * NKV + PAD + t0 + 512],
                        start=(ci == 0), stop=(ci == 3))
                nc.scalar.activation(qT[:, co * NTOK + t0:co * NTOK + t0 + 512],
                                     ps[:, 0:512],
                                     mybir.ActivationFunctionType.Identity,
                                     bias=bq_t[:, co:co + 1])

        # ---- kT (feature-major) ----
        KCH = [512, 512, 512, 512, 64]
        for co in range(_P1):
            t0 = 0
            for w in KCH:
                ps = proj_ps()
                for ci in range(4):
                    nc.tensor.matmul(
                        ps[:, 0:w], wk[ci][:, co * 128:(co + 1) * 128],
                        xT[:, ci * NKV + t0:ci * NKV + t0 + w],
                        start=(ci == 0), stop=(ci == 3))
                nc.scalar.activation(kT[:, co * NKV + t0:co * NKV + t0 + w],
                                     ps[:, 0:w],
                                     mybir.ActivationFunctionType.Identity,
                                     bias=bk_t[:, co:co + 1])
                t0 += w

        # ---- v_tok (token-major, packed [64 v | ones] per head) ----
        for t in range(17 if _P1 else 0):
            r0, r1 = t * 128, min((t + 1) * 128, NKV)
            rows = r1 - r0
            ps = proj_ps()
            for ci in range(4):
                nc.tensor.matmul(
                    ps[:rows, 0:512], xT[:, ci * NKV + r0:ci * NKV + r1],
                    wv[ci][:], start=(ci == 0), stop=(ci == 3))
            vv = v_tok[t].rearrange("p (h y) -> p h y", h=8)
            nc.gpsimd.memset(vv[:, :, 64:65], 1.0)
            nc.vector.scalar_tensor_tensor(
                vv[:rows, :, 0:64],
                ps[:rows, 0:512].rearrange("p (h y) -> p h y", h=8),
                1.0,
                bvB.rearrange("p (h y) -> p h y", h=8)[:rows],
                op0=mybir.AluOpType.mult, op1=mybir.AluOpType.add)

        # ---- attention: per 128-query block ----
        KPH = int(os.environ.get("KPH", "5"))
        for i in range(NB if KPH >= 2 else 0):
            sc = pp_sc.tile([128, 2048], F32, tag="sc")
            # Heads grouped by operand partition base per PSUM bank (a
            # matmul's tile_position row must be uniform within a bank):
            # even heads (base 0) fill bank 0/2, odd heads (base 64) 1/3.
            # Each bank is one accumulation group: 4 head scores writing
            # disjoint quarters, then one additive band-mask matmul
            # (band @ [I I I I]) over the whole bank; out-of-band scores
            # drop to ~-2000 so exp underflows to exactly 0.
            for b in range(2):
                for j in range(4):
                    h = 2 * j + b
                    co, hr = h // 2, (h % 2) * 64
                    nc.tensor.matmul(
                        sc[:, b * 512 + j * 128:b * 512 + (j + 1) * 128],
                        kT[hr:hr + 64, co * NKV + i * 128:co * NKV + i * 128 + 128],
                        qT[hr:hr + 64, co * NTOK + i * 128:co * NTOK + (i + 1) * 128],
                        start=(j == 0), stop=False, skip_group_check=True)
                nc.tensor.matmul(
                    sc[:, b * 512:(b + 1) * 512], bandat[:], identj[:],
                    start=False, stop=True, skip_group_check=True)
            for b in range(2):
                for j in range(4):
                    h = 2 * j + b
                    co, hr = h // 2, (h % 2) * 64
                    c0 = 1024 + b * 512 + j * 64
                    nc.tensor.matmul(
                        sc[0:64, c0:c0 + 64],
                        kT[hr:hr + 64, co * NKV + i * 128 + 128:co * NKV + i * 128 + 192],
                        qT[hr:hr + 64, co * NTOK + i * 128 + 64:co * NTOK + i * 128 + 128],
                        start=(j == 0), stop=False, skip_group_check=True)
                nc.tensor.matmul(
                    sc[0:64, 1024 + b * 512:1280 + b * 512], bandbt[:],
                    identj32[:], start=False, stop=True,
                    skip_group_check=True)
            alpha = sb_a.tile([128, 1536], BF16, tag="alpha")
            for b in range(2):
                nc.scalar.activation(alpha[:, b * 512:(b + 1) * 512],
                                     sc[:, b * 512:(b + 1) * 512],
                                     mybir.ActivationFunctionType.Exp, scale=SCALE)
            for b in range(2):
                nc.scalar.activation(alpha[0:64, 1024 + b * 256:1280 + b * 256],
                                     sc[0:64, 1024 + b * 512:1280 + b * 512],
                                     mybir.ActivationFunctionType.Exp, scale=SCALE)
            if KPH < 3:
                continue

            av = pp_av.tile([128, 1024], F32, tag="av")
            for h in range(8):
                c0 = (h // 4) * 512 + (h % 4) * 65
                ac = (h % 2) * 512 + (h // 2) * 128
                bc = 1024 + (h % 2) * 256 + (h // 2) * 64
                nc.tensor.matmul(
                    av[:, c0:c0 + 65],
                    alpha[:, ac:ac + 128],
                    v_tok[i][:, h * 65:h * 65 + 65],
                    start=True, stop=False, skip_group_check=True)
                nc.tensor.matmul(
                    av[64:128, c0:c0 + 65],
                    alpha[0:64, bc:bc + 64],
                    v_tok[i + 1][0:64, h * 65:h * 65 + 65],
                    start=False, stop=True, skip_group_check=True)
            avv = (av.rearrange("p (a c) -> p a c", a=2)[:, :, 0:260]
                     .rearrange("p a (h y) -> p a h y", h=4))
            rden = sb_o.tile([128, 8], F32, tag="rden")
            nc.vector.reciprocal(rden.rearrange("p (a h) -> p a h", a=2),
                                 avv[:, :, :, 64:65].squeeze(3))
            nc.vector.tensor_scalar_mul(rden[:], rden[:], mq[:, i:i + 1])
            avn = sb_o.tile([128, 512], BF16, tag="avn")
            for a in range(2):
                nc.vector.scalar_tensor_tensor(
                    avn[:, a * 256:(a + 1) * 256]
                       .rearrange("p (h y) -> p h y", h=4),
                    avv[:, a:a + 1, :, 0:64].squeeze(1), 1.0,
                    rden[:, a * 4:(a + 1) * 4].unsqueeze(2)
                        .broadcast_to((128, 4, 64)),
                    op0=mybir.AluOpType.mult, op1=mybir.AluOpType.mult)

            if KPH < 4:
                continue
            tr = pp_tr.tile([128, 512], BF16, tag="tr")
            for ct in range(4):
                nc.tensor.transpose(
                    tr[:, ct * 128:(ct + 1) * 128],
                    avn[:, ct * 128:(ct + 1) * 128],
                    ident[:])
            nc.scalar.activation(
                aT.rearrange("p (a c) -> p a c", a=4)[:, :, i * 128:(i + 1) * 128],
                tr.rearrange("p (a c) -> p a c", a=4),
                mybir.ActivationFunctionType.Copy)

            if KPH < 5:
                continue
            pr = pp_av.tile([128, 1024], F32, tag="av", name="pr")
            for ct in range(4):
                nc.tensor.matmul(
                    pr[:, 0:512],
                    aT[:, ct * NTOK + i * 128:ct * NTOK + (i + 1) * 128],
                    wp[ct][:], start=(ct == 0), stop=False,
                    skip_group_check=True)
            nc.tensor.matmul(
                pr[:, 0:512], mqr[0:1, i * 128:(i + 1) * 128], bpr[:],
                start=False, stop=True, skip_group_check=True)
            ot = sb_o.tile([128, C], F32, tag="ot")
            nc.vector.tensor_copy(ot[:], pr[:, 0:512])
            nc.sync.dma_start(outd[i * 128:(i + 1) * 128, :], ot[:])

    nc.compile()
    return nc


_CACHE: dict = {}


def _get_program() -> bacc.Bacc:
    if "nc" not in _CACHE:
        _CACHE["nc"] = build_program()
    return _CACHE["nc"]


def kernel(x, mask, Wq, bq, Wkv, bkv, Wproj, bproj) -> np.ndarray:
    x = np.asarray(x, np.float32)
    mask = np.asarray(mask, np.float32)
    consts = _mask_consts()
    ident = np.ascontiguousarray(_identity())
    nc = _get_program()

    in_maps = []
    for core in range(8):
        b, h = divmod(core, 2)
        s = h * NTOK
        xc = np.zeros((NKV, C), np.float32)
        mc = np.zeros((NKV,), np.float32)
        lo, hi = max(0, s - PAD), min(T, s + NTOK + PAD)
        xc[lo - (s - PAD):lo - (s - PAD) + hi - lo] = x[b, lo:hi]
        mc[lo - (s - PAD):lo - (s - PAD) + hi - lo] = mask[b, lo:hi]
        in_maps.append({
            "x": xc, "mask": mc,
            "wq": np.asarray(Wq, np.float32), "bq": np.asarray(bq, np.float32),
            "wkv": np.asarray(Wkv, np.float32), "bkv": np.asarray(bkv, np.float32),
            "wproj": np.asarray(Wproj, np.float32),
            "bproj": np.asarray(bproj, np.float32),
            "ident": ident, **consts,
        })

    res = bass_utils.run_bass_kernel_spmd(nc, in_maps, core_ids=list(range(8)))
    out = np.empty((B, T, C), np.float32)
    for core in range(8):
        b, h = divmod(core, 2)
        out[b, h * NTOK:(h + 1) * NTOK] = res.results[core]["out"]
    return out


# revision 25
# speedup vs baseline: 1.2277x; 1.2277x over previous
"""Local (sliding-window) self-attention Bass kernel for 8 TRN2 NeuronCores.

Problem: B=4, T=4096, C=512, H=8 heads, head_dim=64, window=15.
Sharding: 8 cores = batch(4) x seq-halves(2). Each core processes 2048 query
tokens of one batch element; its x chunk carries a 7-token halo on each side
(zero-padded at sequence edges, matching the reference's jnp.pad semantics),
padded to 2080 rows for 128/32 alignment.

v2 dataflow (no DMA transposes; scores computed k-major so alpha never needs
transposing; per-128-token-block batching of ACT/DVE work):
  x chunk --mask*cast--> x_tok bf16 --PE transpose--> xT (feature-major)
  qT/kT = W-stationary matmuls + bias (feature-major)
  v_tok = xT-stationary matmuls + bias, token-major, packed [64 v | 1 ones]
          per head (the ones column makes AV emit the softmax denominator)
  per 128-query block:
    scoresT [k,q] = kT-slice.T @ qT-slice per head (A:128k + B:16k chunks)
    one Exp over the whole block's scores -> alphaT bf16
    one DVE band-mask multiply
    AV token-major: av[q, 65] per head = alphaT.T @ v_aug  (den in col 64)
    reciprocal + one DVE normalize (rden broadcast via 0-stride AP) -> bf16
    4 PE transposes -> attnT feature-major -> one ACT copy -> aT
    proj: aT-stationary matmuls + bias + mask -> DMA out (token-major)
"""

import math
import os
from contextlib import ExitStack

import ml_dtypes
import numpy as np

import concourse.bacc as bacc
import concourse.bass as bass
import concourse.mybir as mybir
import concourse.tile as tile
from concourse import bass_utils

B, T, C, H, WIN = 4, 4096, 512, 8, 15
D = C // H            # 64
PAD = WIN // 2        # 7
NTOK = T // 2         # 2048 query tokens per core
NKV = 2112            # kv rows per core: 7 + 2048 + 7 = 2062, padded to 2112
                      # (the per-block k-tail chunk is 64 wide: 16*128+64)
NB = NTOK // 128      # 16 query blocks
SCALE = math.log(WIN) / D
F32 = mybir.dt.float32
BF16 = mybir.dt.bfloat16
FP8 = mybir.dt.float8e4
WSC = 16.0           # fp8 weight pre-scale; 1/WSC folded into psum->SBUF copies


M0 = 2000.0   # additive score mask; SCALE*M0 ~ 85 so exp underflows to 0


def _mask_consts() -> dict:
    """Additive band masks (transposed, as matmul lhsT) and a replicated
    identity used to broadcast them across the 4 head-columns of a bank.

    A-chunk in-band: k-14 <= q <= k.  B-chunk (k=128+r): q >= 114+r.
    """
    k = np.arange(128)[:, None]
    q = np.arange(128)[None, :]
    a = np.where((q >= k - 14) & (q <= k), 0.0, -M0).astype(np.float32)
    # B tail: k' = 0..63 (k = qb+128+k'), q64 = 0..63 (q = qb+64+q64):
    # in-band iff q >= 114 + k'  <=>  q64 >= 50 + k'
    kp = np.arange(64)[:, None]
    q64 = np.arange(64)[None, :]
    b = np.where(q64 >= 50 + kp, 0.0, -M0).astype(np.float32)
    identj = np.zeros((128, 512), np.float32)
    for j in range(4):
        identj[:, j * 128:(j + 1) * 128] = np.eye(128)
    # K=128 operands (zero-padded rows 64..127) so the B-mask matmul shares
    # the full-array row config of the A-mask (base-mix rule)
    identj32 = np.zeros((128, 256), np.float32)
    for j in range(4):
        identj32[0:64, j * 64:(j + 1) * 64] = np.eye(64)
    return {
        "bandat": np.ascontiguousarray(a.T.astype(ml_dtypes.bfloat16)),
        "bandbt": np.ascontiguousarray(
            np.vstack([b.T, np.zeros((64, 64), np.float32)])
              .astype(ml_dtypes.bfloat16)),
        "identj": np.ascontiguousarray(identj.astype(ml_dtypes.bfloat16)),
        "identj32": np.ascontiguousarray(identj32.astype(ml_dtypes.bfloat16)),
    }


def _identity() -> np.ndarray:
    return np.eye(128, dtype=ml_dtypes.bfloat16)


def build_program() -> bacc.Bacc:
    nc = bacc.Bacc("TRN2", target_bir_lowering=False, debug=False,
                   enable_asserts=False, num_devices=8)

    xd = nc.dram_tensor("x", [NKV, C], F32, kind="ExternalInput").ap()
    maskd = nc.dram_tensor("mask", [NKV], F32, kind="ExternalInput").ap()
    wqd = nc.dram_tensor("wq", [C, C], F32, kind="ExternalInput").ap()
    bqd = nc.dram_tensor("bq", [C], F32, kind="ExternalInput").ap()
    wkvd = nc.dram_tensor("wkv", [C, 2 * C], F32, kind="ExternalInput").ap()
    bkvd = nc.dram_tensor("bkv", [2 * C], F32, kind="ExternalInput").ap()
    wpd = nc.dram_tensor("wproj", [C, C], F32, kind="ExternalInput").ap()
    bpd = nc.dram_tensor("bproj", [C], F32, kind="ExternalInput").ap()
    bandatd = nc.dram_tensor("bandat", [128, 128], BF16, kind="ExternalInput").ap()
    bandbtd = nc.dram_tensor("bandbt", [128, 64], BF16, kind="ExternalInput").ap()
    identjd = nc.dram_tensor("identj", [128, 512], BF16, kind="ExternalInput").ap()
    identj32d = nc.dram_tensor("identj32", [128, 256], BF16, kind="ExternalInput").ap()
    identd = nc.dram_tensor("ident", [128, 128], BF16, kind="ExternalInput").ap()
    outd = nc.dram_tensor("out", [NTOK, C], F32, kind="ExternalOutput").ap()

    with tile.TileContext(nc) as tc, ExitStack() as ctx:
        sb = ctx.enter_context(tc.tile_pool(name="sb", bufs=1))
        sb_ld = ctx.enter_context(tc.tile_pool(name="sb_ld", bufs=3))
        sb_a = ctx.enter_context(tc.tile_pool(name="sb_a", bufs=3))
        sb_o = ctx.enter_context(tc.tile_pool(name="sb_o", bufs=3))
        pp_sc = ctx.enter_context(tc.tile_pool(name="pp_sc", bufs=1, space="PSUM"))
        pp_tr = ctx.enter_context(tc.tile_pool(name="pp_tr", bufs=1, space="PSUM"))
        pp_pr = ctx.enter_context(tc.tile_pool(name="pp_pr", bufs=1, space="PSUM"))
        pp_av = ctx.enter_context(tc.tile_pool(name="pp_av", bufs=1, space="PSUM"))

        # ---- persistent SBUF tensors ----
        xT = sb.tile([128, 4 * NKV], BF16, tag="xT")     # col ci*NKV + t
        qT = sb.tile([128, 4 * NTOK], BF16, tag="qT")    # col co*NTOK + t
        kT = sb.tile([128, 4 * NKV], BF16, tag="kT")     # col co*NKV + t
        aT = sb.tile([128, 4 * NTOK], BF16, tag="aT")    # col ct*NTOK + q
        v_tok = [sb.tile([128, 520], BF16, tag=f"vtok{i}", name=f"vtok{i}")
                 for i in range(17)]                     # col h*65: [64 v | 1]
        bandat = sb.tile([128, 128], BF16, tag="bandat")
        bandbt = sb.tile([128, 64], BF16, tag="bandbt")
        identj = sb.tile([128, 512], BF16, tag="identj")
        identj32 = sb.tile([128, 256], BF16, tag="identj32")
        ident = sb.tile([128, 128], BF16, tag="ident")
        mqr = sb.tile([1, NTOK], BF16, tag="mqr")     # mask row (proj bias fold)
        bpr = sb.tile([1, C], BF16, tag="bpr")        # bproj row
        wq = [sb.tile([128, C], BF16, tag=f"wq{i}", name=f"wq{i}") for i in range(4)]
        wk = [sb.tile([128, C], BF16, tag=f"wk{i}", name=f"wk{i}") for i in range(4)]
        wv = [sb.tile([128, C], BF16, tag=f"wv{i}", name=f"wv{i}") for i in range(4)]
        wp = [sb.tile([128, C], BF16, tag=f"wp{i}", name=f"wp{i}") for i in range(4)]
        bq_t = sb.tile([128, 4], F32, tag="bq")       # per-partition q bias
        bk_t = sb.tile([128, 4], F32, tag="bk")       # per-partition k bias
        bvB = sb.tile([128, C], F32, tag="bvB")       # v bias bcast over partitions
        mq = sb.tile([128, NB], F32, tag="mq")        # query-token mask, per block

        # ---- constants / weights in ----
        nc.sync.dma_start(bandat[:], bandatd)
        nc.sync.dma_start(bandbt[:], bandbtd)
        nc.sync.dma_start(identj[:], identjd)
        nc.sync.dma_start(identj32[:], identj32d)
        nc.sync.dma_start(ident[:], identd)
        mqf = sb_ld.tile([1, NTOK], F32, tag="mqf")
        nc.sync.dma_start(mqf[:], maskd[PAD:PAD + NTOK][None, :])
        nc.vector.tensor_copy(mqr[:], mqf[:])
        bprf = sb_ld.tile([1, C], F32, tag="bprf")
        nc.sync.dma_start(bprf[:], bpd[None, :])
        nc.vector.tensor_copy(bpr[:], bprf[:])
        nc.sync.dma_start(bq_t[:], bqd.rearrange("(a b) -> b a", b=128))
        nc.sync.dma_start(bk_t[:], bkvd[0:C].rearrange("(a b) -> b a", b=128))
        nc.sync.dma_start(bvB[:], bkvd[C:2 * C][None, :].broadcast_to((128, C)))
        nc.sync.dma_start(mq[:], maskd[PAD:PAD + NTOK].rearrange("(a b) -> b a", b=128))
        for ci in range(4):
            wqf = sb_ld.tile([128, C], F32, tag="wld")
            nc.sync.dma_start(wqf[:], wqd[ci * 128:(ci + 1) * 128, :])
            nc.vector.tensor_copy(wq[ci][:], wqf[:])
            wkf = sb_ld.tile([128, 2 * C], F32, tag="wld2")
            nc.sync.dma_start(wkf[:], wkvd[ci * 128:(ci + 1) * 128, :])
            nc.vector.tensor_copy(wk[ci][:], wkf[:, 0:C])
            nc.vector.tensor_copy(wv[ci][:], wkf[:, C:2 * C])
            wpf = sb_ld.tile([128, C], F32, tag="wld")
            nc.sync.dma_start(wpf[:], wpd[ci * 128:(ci + 1) * 128, :])
            nc.vector.tensor_copy(wp[ci][:], wpf[:])

        # ---- x in: mask*cast, then PE-transpose to feature-major ----
        for t in range(17):
            r0, r1 = t * 128, min((t + 1) * 128, NKV)
            rows = r1 - r0
            xf = sb_ld.tile([128, C], F32, tag="xf")
            nc.sync.dma_start(xf[:rows, :], xd[r0:r1, :])
            mrow = sb_ld.tile([128, 1], F32, tag="mrow")
            nc.sync.dma_start(mrow[:rows, :], maskd[r0:r1][:, None])
            xb = sb_ld.tile([128, C], BF16, tag="xb")
            nc.vector.tensor_scalar_mul(xb[:rows, :], xf[:rows, :], mrow[:rows, :])
            xtr = pp_tr.tile([128, 512], BF16, tag="tr", name="xtr")
            for ci in range(4):
                nc.tensor.transpose(
                    xtr[:, ci * 128:ci * 128 + rows],
                    xb[:rows, ci * 128:(ci + 1) * 128],
                    ident[:rows, :rows])
            nc.scalar.activation(
                xT.rearrange("p (a c) -> p a c", a=4)[:, :, r0:r1],
                xtr.rearrange("p (a c) -> p a c", a=4)[:, :, 0:rows],
                mybir.ActivationFunctionType.Copy)

        # Alternate projection-phase PSUM tiles between the two big pools so
        # consecutive chunks double-buffer (each pool alone has bufs=1).
        pcnt = [0]

        def proj_ps():
            pool = pp_sc if pcnt[0] % 2 == 0 else pp_av
            pcnt[0] += 1
            shape = [128, 2048] if pool is pp_sc else [128, 1024]
            return pool.tile(shape, F32, tag="sc" if pool is pp_sc else "av",
                             name=f"pps{pcnt[0]}")

        # ---- qT (feature-major): W stationary, xT moving ----
        _P1 = 4 if int(os.environ.get("KPH", "5")) >= 1 else 0
        for co in range(_P1):
            for ch in range(4):
                t0 = ch * 512
                ps = proj_ps()
                for ci in range(4):
                    nc.tensor.matmul(
                        ps[:, 0:512], wq[ci][:, co * 128:(co + 1) * 128],
                        xT[:, ci * NKV + PAD + t0:ci * NKV + PAD + t0 + 512],
                        start=(ci == 0), stop=(ci == 3))
                nc.scalar.activation(qT[:, co * NTOK + t0:co * NTOK + t0 + 512],
                                     ps[:, 0:512],
                                     mybir.ActivationFunctionType.Identity,
                                     bias=bq_t[:, co:co + 1])

        # ---- kT (feature-major) ----
        KCH = [512, 512, 512, 512, 64]
        for co in range(_P1):
            t0 = 0
            for w in KCH:
                ps = proj_ps()
                for ci in range(4):
                    nc.tensor.matmul(
                        ps[:, 0:w], wk[ci][:, co * 128:(co + 1) * 128],
                        xT[:, ci * NKV + t0:ci * NKV + t0 + w],
                        start=(ci == 0), stop=(ci == 3))
                nc.scalar.activation(kT[:, co * NKV + t0:co * NKV + t0 + w],
                                     ps[:, 0:w],
                                     mybir.ActivationFunctionType.Identity,
                                     bias=bk_t[:, co:co + 1])
                t0 += w

        # ---- v_tok (token-major, packed [64 v | ones] per head) ----
        for t in range(17 if _P1 else 0):
            r0, r1 = t * 128, min((t + 1) * 128, NKV)
            rows = r1 - r0
            ps = proj_ps()
            for ci in range(4):
                nc.tensor.matmul(
                    ps[:rows, 0:512], xT[:, ci * NKV + r0:ci * NKV + r1],
                    wv[ci][:], start=(ci == 0), stop=(ci == 3))
            vv = v_tok[t].rearrange("p (h y) -> p h y", h=8)
            nc.gpsimd.memset(vv[:, :, 64:65], 1.0)
            nc.vector.scalar_tensor_tensor(
                vv[:rows, :, 0:64],
                ps[:rows, 0:512].rearrange("p (h y) -> p h y", h=8),
                1.0,
                bvB.rearrange("p (h y) -> p h y", h=8)[:rows],
                op0=mybir.AluOpType.mult, op1=mybir.AluOpType.add)

        # ---- attention: per 128-query block ----
        KPH = int(os.environ.get("KPH", "5"))
        for i in range(NB if KPH >= 2 else 0):
            sc = pp_sc.tile([128, 2048], F32, tag="sc")
            # Heads grouped by operand partition base per PSUM bank (a
            # matmul's tile_position row must be uniform within a bank):
            # even heads (base 0) fill bank 0/2, odd heads (base 64) 1/3.
            # Each bank is one accumulation group: 4 head scores writing
            # disjoint quarters, then one additive band-mask matmul
            # (band @ [I I I I]) over the whole bank; out-of-band scores
            # drop to ~-2000 so exp underflows to exactly 0.
            for b in range(2):
                for j in range(4):
                    h = 2 * j + b
                    co, hr = h // 2, (h % 2) * 64
                    nc.tensor.matmul(
                        sc[:, b * 512 + j * 128:b * 512 + (j + 1) * 128],
                        kT[hr:hr + 64, co * NKV + i * 128:co * NKV + i * 128 + 128],
                        qT[hr:hr + 64, co * NTOK + i * 128:co * NTOK + (i + 1) * 128],
                        start=(j == 0), stop=False, skip_group_check=True)
                nc.tensor.matmul(
                    sc[:, b * 512:(b + 1) * 512], bandat[:], identj[:],
                    start=False, stop=True, skip_group_check=True)
            for b in range(2):
                for j in range(4):
                    h = 2 * j + b
                    co, hr = h // 2, (h % 2) * 64
                    c0 = 1024 + b * 512 + j * 64
                    nc.tensor.matmul(
                        sc[0:64, c0:c0 + 64],
                        kT[hr:hr + 64, co * NKV + i * 128 + 128:co * NKV + i * 128 + 192],
                        qT[hr:hr + 64, co * NTOK + i * 128 + 64:co * NTOK + i * 128 + 128],
                        start=(j == 0), stop=False, skip_group_check=True)
                nc.tensor.matmul(
                    sc[0:64, 1024 + b * 512:1280 + b * 512], bandbt[:],
                    identj32[:], start=False, stop=True,
                    skip_group_check=True)
            alpha = sb_a.tile([128, 1536], BF16, tag="alpha")
            for b in range(2):
                nc.scalar.activation(alpha[:, b * 512:(b + 1) * 512],
                                     sc[:, b * 512:(b + 1) * 512],
                                     mybir.ActivationFunctionType.Exp, scale=SCALE)
            for b in range(2):
                nc.scalar.activation(alpha[0:64, 1024 + b * 256:1280 + b * 256],
                                     sc[0:64, 1024 + b * 512:1280 + b * 512],
                                     mybir.ActivationFunctionType.Exp, scale=SCALE)
            if KPH < 3:
                continue

            av = pp_av.tile([128, 1024], F32, tag="av")
            for h in range(8):
                c0 = (h // 4) * 512 + (h % 4) * 65
                ac = (h % 2) * 512 + (h // 2) * 128
                bc = 1024 + (h % 2) * 256 + (h // 2) * 64
                nc.tensor.matmul(
                    av[:, c0:c0 + 65],
                    alpha[:, ac:ac + 128],
                    v_tok[i][:, h * 65:h * 65 + 65],
                    start=True, stop=False, skip_group_check=True)
                nc.tensor.matmul(
                    av[64:128, c0:c0 + 65],
                    alpha[0:64, bc:bc + 64],
                    v_tok[i + 1][0:64, h * 65:h * 65 + 65],
                    start=False, stop=True, skip_group_check=True)
            avv = (av.rearrange("p (a c) -> p a c", a=2)[:, :, 0:260]
                     .rearrange("p a (h y) -> p a h y", h=4))
            rden = sb_o.tile([128, 8], F32, tag="rden")
            nc.vector.reciprocal(rden.rearrange("p (a h) -> p a h", a=2),
                                 avv[:, :, :, 64:65].squeeze(3))
            nc.vector.tensor_scalar_mul(rden[:], rden[:], mq[:, i:i + 1])
            avn = sb_o.tile([128, 512], BF16, tag="avn")
            for a in range(2):
                nc.vector.scalar_tensor_tensor(
                    avn[:, a * 256:(a + 1) * 256]
                       .rearrange("p (h y) -> p h y", h=4),
                    avv[:, a:a + 1, :, 0:64].squeeze(1), 1.0,
                    rden[:, a * 4:(a + 1) * 4].unsqueeze(2)
                        .broadcast_to((128, 4, 64)),
                    op0=mybir.AluOpType.mult, op1=mybir.AluOpType.mult)

            if KPH < 4:
                continue
            tr = pp_tr.tile([128, 512], BF16, tag="tr")
            for ct in range(4):
                nc.tensor.transpose(
                    tr[:, ct * 128:(ct + 1) * 128],
                    avn[:, ct * 128:(ct + 1) * 128],
                    ident[:])
            nc.scalar.activation(
                aT.rearrange("p (a c) -> p a c", a=4)[:, :, i * 128:(i + 1) * 128],
                tr.rearrange("p (a c) -> p a c", a=4),
                mybir.ActivationFunctionType.Copy)

            if KPH < 5:
                continue
            pr = pp_pr.tile([128, 512], F32, tag="pr")
            for ct in range(4):
                nc.tensor.matmul(
                    pr[:], aT[:, ct * NTOK + i * 128:ct * NTOK + (i + 1) * 128],
                    wp[ct][:], start=(ct == 0), stop=False,
                    skip_group_check=True)
            nc.tensor.matmul(
                pr[:], mqr[0:1, i * 128:(i + 1) * 128], bpr[:],
                start=False, stop=True, skip_group_check=True)
            ot = sb_o.tile([128, C], F32, tag="ot")
            nc.vector.tensor_copy(ot[:], pr[:])
            nc.sync.dma_start(outd[i * 128:(i + 1) * 128, :], ot[:])

    nc.compile()
    return nc


_CACHE: dict = {}


def _get_program() -> bacc.Bacc:
    if "nc" not in _CACHE:
        _CACHE["nc"] = build_program()
    return _CACHE["nc"]


def kernel(x, mask, Wq, bq, Wkv, bkv, Wproj, bproj) -> np.ndarray:
    x = np.asarray(x, np.float32)
    mask = np.asarray(mask, np.float32)
    consts = _mask_consts()
    ident = np.ascontiguousarray(_identity())
    nc = _get_program()

    in_maps = []
    for core in range(8):
        b, h = divmod(core, 2)
        s = h * NTOK
        xc = np.zeros((NKV, C), np.float32)
        mc = np.zeros((NKV,), np.float32)
        lo, hi = max(0, s - PAD), min(T, s + NTOK + PAD)
        xc[lo - (s - PAD):lo - (s - PAD) + hi - lo] = x[b, lo:hi]
        mc[lo - (s - PAD):lo - (s - PAD) + hi - lo] = mask[b, lo:hi]
        in_maps.append({
            "x": xc, "mask": mc,
            "wq": np.asarray(Wq, np.float32), "bq": np.asarray(bq, np.float32),
            "wkv": np.asarray(Wkv, np.float32), "bkv": np.asarray(bkv, np.float32),
            "wproj": np.asarray(Wproj, np.float32),
            "bproj": np.asarray(bproj, np.float32),
            "ident": ident, **consts,
        })

    res = bass_utils.run_bass_kernel_spmd(nc, in_maps, core_ids=list(range(8)))
    out = np.empty((B, T, C), np.float32)
    for core in range(8):
        b, h = divmod(core, 2)
        out[b, h * NTOK:(h + 1) * NTOK] = res.results[core]["out"]
    return out
